# revision 1
# baseline (speedup 1.0000x reference)
"""Trainium2 kernel for nn_AttentionConstrainedLoss.

Strategy (8 NeuronCores, full inputs in / full output out):
  - The memory-heavy part is the per-grid unbiased variance over D=128 of
    atten_map [B=4, HW=65536, D=128] fp32 (128 MiB).  Sharding: data-parallel
    over B (4 scenes) x sequence-sharded over HW (2 halves) = 8 cores, each
    streaming a 16 MiB chunk and emitting 32768 per-grid variances.
  - On device, variance is computed in a single DVE pass per element with
    bn_stats (per-128-element-chunk count/mean/M2 for even & odd lanes),
    then combined:  M2 = cv_e + cv_o + (n_e*n_o/n)*(m_e-m_o)^2,  var = M2/127.
  - The box -> grid assignment (point-in-rotated-rect over a 0.4 m grid,
    sequential overlap-kill scan, segment means) touches only ~400 cells per
    box (boxes are <= 5 m).  It is exact, tiny, and done on host in fp32
    numpy replicating the reference semantics including scan order and
    argmin tie-breaking.
"""

import numpy as np

# ---------------------------------------------------------------------------
# Problem constants (hardcoded per contract; kernel.py must be self-contained)
# ---------------------------------------------------------------------------
B, M, D = 4, 100, 128
H, W = 256, 256
HW = H * W
N_CORES = 8
HALF = HW // 2  # grid rows per core (sequence shard)
P = 128  # SBUF partitions
TPP = HALF // P  # grid cells per partition per core (256)

_PC_RANGE = np.asarray([-51.2, -51.2, -5.0, 51.2, 51.2, 3.0], dtype=np.float32)
_DIMS = _PC_RANGE[3:] - _PC_RANGE[:3]
_EFF_MIN, _EFF_MAX = np.float32(1.0), np.float32(6.0)

_NC_CACHE = {}
_CFG = {}


def _build_bass_program():
    """Per-core program: atten chunk [32768, 128] f32 -> variance [128, 256] f32.

    Partition layout: grid cell g_local = p * 256 + t  (p = partition,
    t = free index).  Each partition reads 256*128 = 32768 contiguous fp32
    from HBM, so big DMAs stay fully contiguous per partition.

    Structure: 32 DMA blocks of 1024 fp32/partition (512 KiB each), each its
    own SBUF tile so Tile's per-tile dependency tracking pipelines at block
    granularity.  The first/last blocks are further split into 4 quarter
    tiles (128 KiB DMAs) to shorten the pipeline ramp and tail.  Each block
    holds 8 grid-cell chunks of 128 values; most go to DVE (one-pass
    bn_stats), a fixed per-block count goes to ScalarE (Copy+Square with
    accum) to keep both engines below the ~47 us DMA roofline.  Stats are
    accumulated per half so the first half's variance combine + store hide
    under the second half's streaming.
    """
    import concourse.bacc as bacc
    import concourse.mybir as mybir
    from concourse import tile

    f32 = mybir.dt.float32

    nc = bacc.Bacc("TRN2", target_bir_lowering=False, debug=False)
    atten = nc.dram_tensor("atten", [HALF, D], f32, kind="ExternalInput")
    v_out = nc.dram_tensor("v_out", [P, TPP], f32, kind="ExternalOutput")

    # [128, 32768] view: partition p <- rows [p*256, (p+1)*256), contiguous.
    av = atten[:, :].rearrange("(p t) d -> p (t d)", p=P)

    NBLK = int(_CFG.get("nblk", 32))
    BW = (TPP * D) // NBLK  # fp32 per partition per block (1024)
    CPB = BW // D  # chunks per block (8)
    # section boundaries: epilogue (combine + store) emitted per section;
    # the small last section keeps the end-of-kernel combine tiny
    SECTIONS = _CFG.get("sections", [(0, 13), (13, 29), (29, 32)])

    # ACT chunk count per block (trailing chunks of the block).  Steady state
    # per 1456 ns block DMA: 7 bn_stats on DVE (1358 ns) + 1 chunk on ACT
    # (978 ns) both fit; the last two blocks are all-DVE to avoid ACT's
    # latency in the kernel tail.
    act_plan = _CFG.get("act_plan")
    if act_plan is None:
        act_plan = [2] * 13 + [1] * 19
    assert len(act_plan) == NBLK

    mult, add = mybir.AluOpType.mult, mybir.AluOpType.add

    def combine_dve(st5, v4, tmp_pool, s0, s1, c0, c1):
        """v[s0:s1, c0:c1] = (cv_e + cv_o + 32*(m_e-m_o)^2)/127 from bn_stats."""
        ns, nch = s1 - s0, c1 - c0
        m_e = st5[:, s0:s1, c0:c1, 1]
        cv_e = st5[:, s0:s1, c0:c1, 2]
        m_o = st5[:, s0:s1, c0:c1, 4]
        cv_o = st5[:, s0:s1, c0:c1, 5]
        t_d = tmp_pool.tile([P, ns * nch], f32, tag="t_d")
        t_c = tmp_pool.tile([P, ns * nch], f32, tag="t_c")
        t_d3 = t_d[:].rearrange("p (s c) -> p s c", c=nch)
        t_c3 = t_c[:].rearrange("p (s c) -> p s c", c=nch)
        nc.vector.tensor_sub(out=t_d3, in0=m_e, in1=m_o)
        nc.vector.tensor_tensor(out=t_d[:], in0=t_d[:], in1=t_d[:], op=mult)
        nc.vector.tensor_add(out=t_c3, in0=cv_e, in1=cv_o)
        nc.vector.scalar_tensor_tensor(
            out=t_c[:], in0=t_d[:], scalar=32.0, in1=t_c[:], op0=mult, op1=add
        )
        nc.vector.tensor_scalar_mul(v4[:, s0:s1, c0:c1], t_c3, float(1.0 / 127.0))

    with tile.TileContext(nc) as tc:
        with (
            tc.tile_pool(name="io", bufs=int(_CFG.get("bufs", 10))) as io_pool,
            tc.tile_pool(name="scr", bufs=2) as scr_pool,
            tc.tile_pool(name="acc", bufs=1) as acc_pool,
            tc.tile_pool(name="tmp", bufs=2) as tmp_pool,
        ):
            secs = []
            for si, (b0, b1) in enumerate(SECTIONS):
                nb = b1 - b0
                na_s = sum(act_plan[b] for b in range(b0, b1))
                stats_s = acc_pool.tile([P, nb * CPB * 6], f32, tag=f"stats{si}")
                sums_s = acc_pool.tile([P, max(na_s, 1)], f32, tag=f"sums{si}")
                sumsq_s = acc_pool.tile([P, max(na_s, 1)], f32, tag=f"sumsq{si}")
                vtile_s = acc_pool.tile([P, nb * CPB], f32, tag=f"vtile{si}")
                secs.append(
                    dict(
                        b0=b0,
                        b1=b1,
                        nb=nb,
                        na=na_s,
                        stats=stats_s,
                        sums=sums_s,
                        sumsq=sumsq_s,
                        vtile=vtile_s,
                        st5=stats_s[:].rearrange(
                            "p (s c u) -> p s c u", c=CPB, u=6
                        ),
                        v4=vtile_s[:].rearrange("p (s c) -> p s c", c=CPB),
                    )
                )

            def emit_sec_epilogue(si):
                sd = secs[si]
                st5, v4, nb = sd["st5"], sd["v4"], sd["nb"]
                # group contiguous blocks with equal DVE-chunk count
                runs = []  # [start, end, n_dve]
                for i, b in enumerate(range(sd["b0"], sd["b1"])):
                    nd = CPB - act_plan[b]
                    if runs and runs[-1][2] == nd and runs[-1][1] == i:
                        runs[-1][1] = i + 1
                    else:
                        runs.append([i, i + 1, nd])
                base_nd = min(r[2] for r in runs)
                combine_dve(st5, v4, tmp_pool, 0, nb, 0, base_nd)
                for s0, s1, nd in runs:
                    if nd > base_nd:
                        combine_dve(st5, v4, tmp_pool, s0, s1, base_nd, nd)
                if sd["na"] > 0:
                    # ACT chunks: var = (sumsq - sum^2/128) / 127
                    t_u = tmp_pool.tile([P, sd["na"]], f32, tag="t_u")
                    nc.vector.tensor_tensor(
                        out=t_u[:], in0=sd["sums"][:], in1=sd["sums"][:], op=mult
                    )
                    nc.vector.scalar_tensor_tensor(
                        out=t_u[:],
                        in0=t_u[:],
                        scalar=float(-1.0 / 128.0),
                        in1=sd["sumsq"][:],
                        op0=mult,
                        op1=add,
                    )
                    off = 0
                    i = 0
                    while i < nb:
                        na = act_plan[sd["b0"] + i]
                        if na == 0:
                            i += 1
                            continue
                        j = i
                        while j + 1 < nb and act_plan[sd["b0"] + j + 1] == na:
                            j += 1
                        ns = j - i + 1
                        nc.vector.tensor_scalar_mul(
                            v4[:, i : j + 1, CPB - na : CPB],
                            t_u[:, off : off + ns * na].rearrange(
                                "p (s c) -> p s c", c=na
                            ),
                            float(1.0 / 127.0),
                        )
                        off += ns * na
                        i = j + 1
                nc.sync.dma_start(
                    out=v_out[:, sd["b0"] * CPB : sd["b1"] * CPB],
                    in_=sd["vtile"][:],
                )

            for si, (b0, b1) in enumerate(SECTIONS):
                sd = secs[si]
                for b in range(b0, b1):
                    b_local = b - b0
                    n_act = act_plan[b]
                    n_dve = CPB - n_act
                    ca0 = sum(act_plan[x] for x in range(b0, b))

                    if b == NBLK - 1:
                        # half-split the final block: its first chunks compute
                        # while the last 512 KiB half is still in flight
                        HW2 = BW // 2
                        cph = HW2 // D
                        parts = []
                        for q in range(2):
                            qt = io_pool.tile([P, HW2], f32, tag="slabq")
                            nc.sync.dma_start(
                                out=qt[:],
                                in_=av[
                                    :, b * BW + q * HW2 : b * BW + (q + 1) * HW2
                                ],
                            )
                            parts.append(qt)

                        def chunk_ap(k, parts=parts, cph=cph):
                            return parts[k // cph][
                                :, (k % cph) * D : (k % cph + 1) * D
                            ]
                    else:
                        blk = io_pool.tile([P, BW], f32, tag="slab")
                        nc.sync.dma_start(
                            out=blk[:], in_=av[:, b * BW : (b + 1) * BW]
                        )

                        def chunk_ap(k, blk=blk):
                            return blk[:, k * D : (k + 1) * D]

                    for k in range(n_dve):
                        t = b_local * CPB + k
                        nc.vector.bn_stats(
                            out=sd["stats"][:, t * 6 : (t + 1) * 6],
                            in_=chunk_ap(k),
                        )
                    for j in range(n_act):
                        ca = ca0 + j
                        chunk = chunk_ap(n_dve + j)
                        scr = scr_pool.tile([P, D], f32, tag="scr")
                        nc.scalar.activation(
                            out=scr[:],
                            in_=chunk,
                            func=mybir.ActivationFunctionType.Copy,
                            accum_out=sd["sums"][:, ca : ca + 1],
                        )
                        scr2 = scr_pool.tile([P, D], f32, tag="scr")
                        nc.scalar.activation(
                            out=scr2[:],
                            in_=chunk,
                            func=mybir.ActivationFunctionType.Square,
                            accum_out=sd["sumsq"][:, ca : ca + 1],
                        )
                emit_sec_epilogue(si)

    nc.compile()
    return nc


def _get_nc():
    if "nc" not in _NC_CACHE:
        _NC_CACHE["nc"] = _build_bass_program()
    return _NC_CACHE["nc"]


def _device_variance(atten_map: np.ndarray, trace: bool = False):
    """Run the SPMD variance kernel on 8 cores. Returns v [B, HW] f32 (+ results obj)."""
    from concourse.bass_utils import run_bass_kernel_spmd

    nc = _get_nc()
    in_maps = []
    for c in range(N_CORES):
        b, h = c // 2, c % 2
        # slice BEFORE materializing so jax-array inputs transfer in 16 MiB
        # per-core pieces (large single device->host copies can fail)
        chunk = atten_map[b, h * HALF : (h + 1) * HALF, :]
        chunk = np.ascontiguousarray(np.asarray(chunk), dtype=np.float32)
        in_maps.append({"atten": chunk})
    res = run_bass_kernel_spmd(nc, in_maps, list(range(N_CORES)), trace=trace)
    v = np.empty((B, HW), dtype=np.float32)
    for c in range(N_CORES):
        b, h = c // 2, c % 2
        v[b, h * HALF : (h + 1) * HALF] = res.results[c]["v_out"].reshape(HALF)
    return v, res


# ---------------------------------------------------------------------------
# Host-side box logic (exact fp32 replication of the reference semantics)
# ---------------------------------------------------------------------------
def _grid_axis_vals():
    gx = (np.arange(W, dtype=np.float32) + np.float32(0.5)) / np.float32(W) * _DIMS[
        0
    ] + _PC_RANGE[0]
    gy = (np.arange(H, dtype=np.float32) + np.float32(0.5)) / np.float32(H) * _DIMS[
        1
    ] + _PC_RANGE[1]
    return gx, gy


_CORNERS_NORM = np.asarray(
    [[-0.5, -0.5], [-0.5, 0.5], [0.5, 0.5], [0.5, -0.5]], dtype=np.float32
)


def _scene_loss(v: np.ndarray, boxes: np.ndarray, gx: np.ndarray, gy: np.ndarray):
    centers = boxes[:, :2]
    lw = boxes[:, 3:5]
    angles = boxes[:, 6]
    ratio_l = np.clip(_DIMS[0] / np.float32(W) / lw[:, 0], _EFF_MIN, _EFF_MAX)
    ratio_w = np.clip(_DIMS[1] / np.float32(H) / lw[:, 1], _EFF_MIN, _EFF_MAX)
    eff = np.stack([lw[:, 0] * ratio_l, lw[:, 1] * ratio_w], axis=1)
    corners = eff[:, None, :] * _CORNERS_NORM  # [M, 4, 2]
    c = np.cos(angles)[:, None]
    s = np.sin(angles)[:, None]
    rx = corners[..., 0] * c + corners[..., 1] * s
    ry = -corners[..., 0] * s + corners[..., 1] * c
    corners = np.stack([rx, ry], axis=-1) + centers[:, None, :]  # [M, 4, 2]
    edges = np.roll(corners, -1, axis=1) - corners

    # exact argmin (first-index tie-break) of d2 over the full grid, as in ref
    d2 = (gx[None, None, :] - centers[:, 0:1, None]) ** 2 + (
        gy[None, :, None] - centers[:, 1:2, None]
    ) ** 2  # [M, H, W] f32
    nearest_g = np.argmin(d2.reshape(M, HW), axis=1)

    flag = np.full(HW, -1, dtype=np.int32)
    for i in range(M):
        cmin, cmax = corners[i, :, 0].min(), corners[i, :, 0].max()
        rmin, rmax = corners[i, :, 1].min(), corners[i, :, 1].max()
        c0 = max(0, int(np.searchsorted(gx, cmin)) - 1)
        c1 = min(W, int(np.searchsorted(gx, cmax)) + 1)
        r0 = max(0, int(np.searchsorted(gy, rmin)) - 1)
        r1 = min(H, int(np.searchsorted(gy, rmax)) + 1)
        dx = gx[None, None, c0:c1] - corners[i, :, 0][:, None, None]
        dy = gy[None, r0:r1, None] - corners[i, :, 1][:, None, None]
        cross = (
            edges[i, :, 0][:, None, None] * dy - edges[i, :, 1][:, None, None] * dx
        )
        inside = np.all(cross >= 0, axis=0) | np.all(cross <= 0, axis=0)
        rr, cc = np.nonzero(inside)
        gidx = (rr + r0).astype(np.int64) * W + (cc + c0)
        gidx = np.union1d(gidx, np.asarray([nearest_g[i]]))
        cur = flag[gidx]
        flag[gidx] = np.where(cur == -1, np.int32(i), np.int32(-1))

    sums = np.zeros(M, dtype=np.float32)
    cnts = np.zeros(M, dtype=np.float32)
    msk = flag >= 0
    np.add.at(sums, flag[msk], v[msk])
    np.add.at(cnts, flag[msk], np.float32(1.0))
    valid = cnts > 0
    box_mean = sums / np.maximum(cnts, np.float32(1.0))
    loss = -np.sum(box_mean[valid], dtype=np.float32)
    return loss, np.float32(np.sum(valid))


def _host_reduce(v: np.ndarray, gt_bboxes: np.ndarray):
    gx, gy = _grid_axis_vals()
    losses = np.zeros(B, dtype=np.float32)
    nums = np.zeros(B, dtype=np.float32)
    for b in range(B):
        losses[b], nums[b] = _scene_loss(
            v[b], np.asarray(gt_bboxes[b], dtype=np.float32), gx, gy
        )
    var_loss = np.sum(losses, dtype=np.float32)
    var_pos_num = np.maximum(np.sum(nums, dtype=np.float32), np.float32(1.0))
    return np.asarray(np.float32(var_loss / var_pos_num))


def kernel(atten_map: np.ndarray, gt_bboxes: np.ndarray, gt_labels: np.ndarray):
    gt_bboxes = np.asarray(gt_bboxes, dtype=np.float32)
    v, _ = _device_variance(atten_map)
    return _host_reduce(v, gt_bboxes)



# revision 3
# speedup vs baseline: 1.0448x; 1.0448x over previous
"""Trainium2 kernel for nn_AttentionConstrainedLoss.

Strategy (8 NeuronCores, full inputs in / full output out):
  - The memory-heavy part is the per-grid unbiased variance over D=128 of
    atten_map [B=4, HW=65536, D=128] fp32 (128 MiB).  Sharding: data-parallel
    over B (4 scenes) x sequence-sharded over HW (2 halves) = 8 cores, each
    streaming a 16 MiB chunk and emitting 32768 per-grid M2 values (the
    1/127 unbiased-variance scale is applied on host).
  - DVE computes per-grid stats with multi-chunk bn_stats ([128, 4, 128] ->
    [128, 4, 6]; 4 grid cells per instruction amortizes the 58-cycle SBUF
    access), then a 4-op combine per section:
      M2 = cv_e + cv_o + 32*(m_e - m_o)^2.
    DVE totals ~41us against the ~47us DMA roofline, so it never backlogs.
  - Input streams as 28 blocks of 8 cells/partition plus a tapered tail
    (4,4,4,4,3,3,2,2,2,2,2): each tail piece's bn_stats fits inside the next
    piece's DMA+sem window, so the last cell's stats complete ~1.2us after
    the final byte lands (the 900ns DMA-sem propagation is the floor).
  - Section combines are placed so no combine lands at the taper start
    (which would stall the endgame chain); the final combine covers
    [200,256) and is followed by a single dma_start of the whole
    vtile -> v_out, so exactly one store sits in the kernel tail.
  - The box -> grid assignment (point-in-rotated-rect over a 0.4 m grid,
    sequential overlap-kill scan, segment means) touches only ~400 cells per
    box (boxes are <= 5 m).  It is exact, tiny, and done on host in fp32
    numpy replicating the reference semantics including scan order and
    argmin tie-breaking.
"""

import numpy as np

# ---------------------------------------------------------------------------
# Problem constants (hardcoded per contract; kernel.py must be self-contained)
# ---------------------------------------------------------------------------
B, M, D = 4, 100, 128
H, W = 256, 256
HW = H * W
N_CORES = 8
HALF = HW // 2  # grid rows per core (sequence shard)
P = 128  # SBUF partitions
TPP = HALF // P  # grid cells per partition per core (256)

_PC_RANGE = np.asarray([-51.2, -51.2, -5.0, 51.2, 51.2, 3.0], dtype=np.float32)
_DIMS = _PC_RANGE[3:] - _PC_RANGE[:3]
_EFF_MIN, _EFF_MAX = np.float32(1.0), np.float32(6.0)

_NC_CACHE = {}
_CFG = {}


def _piece_plan():
    """DMA piece sizes in grid cells per partition (sums to TPP)."""
    taper = _CFG.get("taper", [4, 4, 4, 4, 3, 3, 2, 2, 2, 2, 2])
    body = (TPP - sum(taper)) // 8
    return [8] * body + taper


def _build_bass_program():
    """Per-core program: atten chunk [32768, 128] f32 -> M2 [128, 256] f32.

    Partition layout: grid cell g_local = p * 256 + t  (p = partition,
    t = free index).  Each partition reads 256*128 = 32768 contiguous fp32
    from HBM, so DMA descriptors stay fully contiguous per partition.
    """
    import concourse.bacc as bacc
    import concourse.mybir as mybir
    from concourse import tile

    f32 = mybir.dt.float32
    mult, add = mybir.AluOpType.mult, mybir.AluOpType.add

    nc = bacc.Bacc("TRN2", target_bir_lowering=False, debug=False)
    atten = nc.dram_tensor("atten", [HALF, D], f32, kind="ExternalInput")
    v_out = nc.dram_tensor("v_out", [P, TPP], f32, kind="ExternalOutput")

    # [128, 32768] view: partition p <- rows [p*256, (p+1)*256), contiguous.
    av = atten[:, :].rearrange("(p t) d -> p (t d)", p=P)

    pieces = _piece_plan()
    assert sum(pieces) == TPP
    # combine-section boundaries (cells); the last section must cover the
    # whole taper plus enough body blocks that no combine is emitted at the
    # taper start (it would stall the endgame bn_stats chain)
    sections = _CFG.get("sections", [0, 128, 200, TPP])
    n_sec = len(sections) - 1

    with tile.TileContext(nc) as tc:
        with (
            tc.tile_pool(name="io", bufs=int(_CFG.get("bufs", 10))) as io_pool,
            tc.tile_pool(name="acc", bufs=1) as acc_pool,
            tc.tile_pool(name="tmp", bufs=2) as tmp_pool,
        ):
            stats = acc_pool.tile([P, TPP * 6], f32, tag="stats")
            st3 = stats[:].rearrange("p (c u) -> p c u", u=6)
            vtile = acc_pool.tile([P, TPP], f32, tag="vtile")

            def emit_combine(si):
                # vtile[a:b] = cv_e + cv_o + 32*(m_e - m_o)^2   (M2; host /127)
                a, b = sections[si], sections[si + 1]
                nch = b - a
                m_e = st3[:, a:b, 1]
                cv_e = st3[:, a:b, 2]
                m_o = st3[:, a:b, 4]
                cv_o = st3[:, a:b, 5]
                t_d = tmp_pool.tile([P, nch], f32, tag="t_d")
                t_c = tmp_pool.tile([P, nch], f32, tag="t_c")
                nc.vector.tensor_sub(out=t_d[:], in0=m_e, in1=m_o)
                nc.vector.tensor_tensor(
                    out=t_d[:], in0=t_d[:], in1=t_d[:], op=mult
                )
                nc.vector.tensor_add(out=t_c[:], in0=cv_e, in1=cv_o)
                nc.vector.scalar_tensor_tensor(
                    out=vtile[:, a:b],
                    in0=t_d[:],
                    scalar=32.0,
                    in1=t_c[:],
                    op0=mult,
                    op1=add,
                )

            sec_i = 0
            chunk = 0
            for n in pieces:
                slab = io_pool.tile([P, n * D], f32, tag=f"slab{n}")
                nc.sync.dma_start(
                    out=slab[:], in_=av[:, chunk * D : (chunk + n) * D]
                )
                off = 0
                while off < n:
                    s = min(4, n - off)
                    c0 = chunk + off
                    nc.vector.bn_stats(
                        out=st3[:, c0 : c0 + s, :],
                        in_=slab[:, off * D : (off + s) * D].rearrange(
                            "p (s d) -> p s d", d=D
                        ),
                    )
                    off += s
                chunk += n
                while sec_i < n_sec and chunk >= sections[sec_i + 1]:
                    emit_combine(sec_i)
                    sec_i += 1

            # single store of the full result; waits only the final combine
            nc.sync.dma_start(out=v_out[:, :], in_=vtile[:])

    nc.compile()
    return nc


def _get_nc():
    if "nc" not in _NC_CACHE:
        _NC_CACHE["nc"] = _build_bass_program()
    return _NC_CACHE["nc"]


def _device_variance(atten_map: np.ndarray, trace: bool = False):
    """Run the SPMD kernel on 8 cores. Returns per-grid M2 [B, HW] f32
    (unbiased variance times 127; scaled on host)."""
    from concourse.bass_utils import run_bass_kernel_spmd

    nc = _get_nc()
    in_maps = []
    for c in range(N_CORES):
        b, h = c // 2, c % 2
        # slice BEFORE materializing so jax-array inputs transfer in 16 MiB
        # per-core pieces (large single device->host copies can fail)
        chunk = atten_map[b, h * HALF : (h + 1) * HALF, :]
        chunk = np.ascontiguousarray(np.asarray(chunk), dtype=np.float32)
        in_maps.append({"atten": chunk})
    res = run_bass_kernel_spmd(nc, in_maps, list(range(N_CORES)), trace=trace)
    v = np.empty((B, HW), dtype=np.float32)
    for c in range(N_CORES):
        b, h = c // 2, c % 2
        v[b, h * HALF : (h + 1) * HALF] = res.results[c]["v_out"].reshape(HALF)
    return v, res


# ---------------------------------------------------------------------------
# Host-side box logic (exact fp32 replication of the reference semantics)
# ---------------------------------------------------------------------------
def _grid_axis_vals():
    gx = (np.arange(W, dtype=np.float32) + np.float32(0.5)) / np.float32(W) * _DIMS[
        0
    ] + _PC_RANGE[0]
    gy = (np.arange(H, dtype=np.float32) + np.float32(0.5)) / np.float32(H) * _DIMS[
        1
    ] + _PC_RANGE[1]
    return gx, gy


_CORNERS_NORM = np.asarray(
    [[-0.5, -0.5], [-0.5, 0.5], [0.5, 0.5], [0.5, -0.5]], dtype=np.float32
)


def _scene_loss(v: np.ndarray, boxes: np.ndarray, gx: np.ndarray, gy: np.ndarray):
    centers = boxes[:, :2]
    lw = boxes[:, 3:5]
    angles = boxes[:, 6]
    ratio_l = np.clip(_DIMS[0] / np.float32(W) / lw[:, 0], _EFF_MIN, _EFF_MAX)
    ratio_w = np.clip(_DIMS[1] / np.float32(H) / lw[:, 1], _EFF_MIN, _EFF_MAX)
    eff = np.stack([lw[:, 0] * ratio_l, lw[:, 1] * ratio_w], axis=1)
    corners = eff[:, None, :] * _CORNERS_NORM  # [M, 4, 2]
    c = np.cos(angles)[:, None]
    s = np.sin(angles)[:, None]
    rx = corners[..., 0] * c + corners[..., 1] * s
    ry = -corners[..., 0] * s + corners[..., 1] * c
    corners = np.stack([rx, ry], axis=-1) + centers[:, None, :]  # [M, 4, 2]
    edges = np.roll(corners, -1, axis=1) - corners

    # exact argmin (first-index tie-break) of d2 over the full grid, as in ref
    d2 = (gx[None, None, :] - centers[:, 0:1, None]) ** 2 + (
        gy[None, :, None] - centers[:, 1:2, None]
    ) ** 2  # [M, H, W] f32
    nearest_g = np.argmin(d2.reshape(M, HW), axis=1)

    flag = np.full(HW, -1, dtype=np.int32)
    for i in range(M):
        cmin, cmax = corners[i, :, 0].min(), corners[i, :, 0].max()
        rmin, rmax = corners[i, :, 1].min(), corners[i, :, 1].max()
        c0 = max(0, int(np.searchsorted(gx, cmin)) - 1)
        c1 = min(W, int(np.searchsorted(gx, cmax)) + 1)
        r0 = max(0, int(np.searchsorted(gy, rmin)) - 1)
        r1 = min(H, int(np.searchsorted(gy, rmax)) + 1)
        dx = gx[None, None, c0:c1] - corners[i, :, 0][:, None, None]
        dy = gy[None, r0:r1, None] - corners[i, :, 1][:, None, None]
        cross = (
            edges[i, :, 0][:, None, None] * dy - edges[i, :, 1][:, None, None] * dx
        )
        inside = np.all(cross >= 0, axis=0) | np.all(cross <= 0, axis=0)
        rr, cc = np.nonzero(inside)
        gidx = (rr + r0).astype(np.int64) * W + (cc + c0)
        gidx = np.union1d(gidx, np.asarray([nearest_g[i]]))
        cur = flag[gidx]
        flag[gidx] = np.where(cur == -1, np.int32(i), np.int32(-1))

    sums = np.zeros(M, dtype=np.float32)
    cnts = np.zeros(M, dtype=np.float32)
    msk = flag >= 0
    np.add.at(sums, flag[msk], v[msk])
    np.add.at(cnts, flag[msk], np.float32(1.0))
    sums *= np.float32(1.0 / 127.0)  # device emits M2; unbiased var = M2/127
    valid = cnts > 0
    box_mean = sums / np.maximum(cnts, np.float32(1.0))
    loss = -np.sum(box_mean[valid], dtype=np.float32)
    return loss, np.float32(np.sum(valid))


def _host_reduce(v: np.ndarray, gt_bboxes: np.ndarray):
    gx, gy = _grid_axis_vals()
    losses = np.zeros(B, dtype=np.float32)
    nums = np.zeros(B, dtype=np.float32)
    for b in range(B):
        losses[b], nums[b] = _scene_loss(
            v[b], np.asarray(gt_bboxes[b], dtype=np.float32), gx, gy
        )
    var_loss = np.sum(losses, dtype=np.float32)
    var_pos_num = np.maximum(np.sum(nums, dtype=np.float32), np.float32(1.0))
    return np.asarray(np.float32(var_loss / var_pos_num))


def kernel(atten_map: np.ndarray, gt_bboxes: np.ndarray, gt_labels: np.ndarray):
    gt_bboxes = np.asarray(gt_bboxes, dtype=np.float32)
    v, _ = _device_variance(atten_map)
    return _host_reduce(v, gt_bboxes)
